# revision 9
# baseline (speedup 1.0000x reference)
"""Gemma3 sliding-window attention decode step (B=32, T=1) on 8 NeuronCores.

Sharding: tensor-parallel by KV head. Core c owns KV head c and Q heads
2c, 2c+1. Wq/Wk/Wv are column-sharded, Wo row-sharded; the host sums the 8
partial outputs (the "all-reduce") and scatters the new k/v rows into the
returned caches.

Per-core device work:
  qkv = x @ [Wq_c | Wk_c | Wv_c]          (fp32r matmuls, PSUM accum)
  per-head RMS-norm + NeoX rope on q, k   (norm weights + rope cos/sin are
                                           folded into 4 host-precomputed
                                           per-dim vectors)
  scores = q @ K_c^T  (K^T streamed from DRAM in a host-transposed layout,
                       new k column inserted on-chip)
  probs  = softmax(scores)                (max-subtracted, normalized on-chip)
  attn   = probs @ V_c (V streamed s-major, new v row inserted on-chip)
  y_c    = attn @ Wo_c

Memory layout is chosen so every large DMA is >=2MB with >=16KB contiguous
per partition. Matmuls run as float32r (full-rate fp32 feed mode).
"""

import numpy as np

import concourse.bass as bass
import concourse.tile as tile
from concourse import mybir
from concourse.bass_utils import run_bass_kernel_spmd
from concourse.masks import make_identity
from concourse.vector_clock import ScopedClock

N_CORES = 8
B = 32
HIDDEN = 3840
NUM_HEADS = 16
NUM_KV_HEADS = 8
HEAD_DIM = 256
S = 1024  # sliding window
ROPE_BASE = 10000.0
SCALE = HEAD_DIM ** -0.5
EPS = 1e-6

QH = NUM_HEADS // N_CORES      # 2 q heads per core
QCOLS = QH * HEAD_DIM          # 512
KT = HIDDEN // 128             # 30 contraction chunks
GB = 2                         # batches per KV group
NG = B // GB                   # 16 groups
WG = 5                         # Wqkv k-chunks per DMA group
F32 = mybir.dt.float32
F32R = mybir.dt.float32r

_n = [0]


def _carrier(engine, wait):
    ev = mybir.InstEventSemaphore(name=f"waitfix-{_n[0]}", ins=[], outs=[])
    _n[0] += 1
    ev.engine = engine
    ev.sync_info = mybir.SyncInfo(on_wait=[wait], on_update=[])
    return ev


class WaitSplitTileContext(tile.TileContext):
    """This walrus build rejects >1 sync wait per instruction. Hoist excess
    waits onto single-wait EventSemaphore carriers in the same engine stream
    (same program point), and split the tail drain into single-wait drains."""

    def _lower_ordered_insts(self, ordered):
        for insts in ordered.values():
            new = []
            for inst in insts:
                si = inst.sync_info
                if si is not None and len(si.on_wait) > 1:
                    waits = list(si.on_wait)
                    for w in waits[:-1]:
                        new.append(_carrier(inst.engine, w))
                    inst.sync_info = mybir.SyncInfo(
                        on_wait=[waits[-1]], on_update=list(si.on_update)
                    )
                new.append(inst)
            insts[:] = new
        return super()._lower_ordered_insts(ordered)

    def _drain_and_barrier(self, tick_clock, wait_clock):
        drain_inst = self.nc.sync.drain()
        wait_clock.add_sem_waits(
            drain_inst.ins, ScopedClock({None: tick_clock.global_clock})
        )
        si = drain_inst.ins.sync_info
        if si is not None and len(si.on_wait) > 1:
            waits = list(si.on_wait)
            drain_inst.ins.sync_info = mybir.SyncInfo(
                on_wait=[waits[0]], on_update=list(si.on_update)
            )
            for w in waits[1:]:
                d2 = self.nc.sync.drain()
                d2.ins.sync_info = mybir.SyncInfo(on_wait=[w], on_update=[])

        self.nc.all_engine_barrier()
        assert self.sems is not None
        popped = self.nc._tile_sem_poison_stack.pop()
        assert popped is self._sem_poison
        self.nc.clear_and_free_semaphores(list(self.sems.allocated().values()))
        self.nc.all_engine_barrier()


def build_kernel(write_idx: int):
    nc = bass.Bass("TRN2", target_bir_lowering=False, debug=False)

    xT = nc.dram_tensor("xT", [128, KT * B], F32, kind="ExternalInput").ap()
    aux = nc.dram_tensor("aux", [B, 8 * 128], F32, kind="ExternalInput").ap()
    Wr = nc.dram_tensor("Wr", [128, KT * 1024], F32, kind="ExternalInput").ap()
    Wor = nc.dram_tensor("Wor", [128, 4 * HIDDEN], F32, kind="ExternalInput").ap()
    KTc = nc.dram_tensor("KTc", [NG, 128, GB * 2 * S], F32R, kind="ExternalInput").ap()
    Vrc = nc.dram_tensor("Vrc", [NG, 128, GB * 8 * HEAD_DIM], F32R, kind="ExternalInput").ap()

    y = nc.dram_tensor("y", [B, HIDDEN], F32, kind="ExternalOutput").ap()
    knew = nc.dram_tensor("knew", [B, HEAD_DIM], F32, kind="ExternalOutput").ap()
    vnew = nc.dram_tensor("vnew", [B, HEAD_DIM], F32, kind="ExternalOutput").ap()

    st_w, p_w = divmod(write_idx, 128)  # V-insert position

    with WaitSplitTileContext(nc) as tc:
        with (
            tc.tile_pool(name="const", bufs=1) as constp,
            tc.tile_pool(name="wq", bufs=2) as wqp,
            tc.tile_pool(name="kv", bufs=2) as kvp,
            tc.tile_pool(name="work", bufs=2) as work,
            tc.tile_pool(name="pbig", bufs=2, space="PSUM") as pbig,
            tc.tile_pool(name="psmall", bufs=2, space="PSUM") as psmall,
            tc.tile_pool(name="pattn", bufs=2, space="PSUM") as pattn,
        ):
            ident = constp.tile([128, 128], F32)
            make_identity(nc, ident[:])
            eps_t = constp.tile([B, 1], F32)
            nc.vector.memset(eps_t[:], EPS)

            xTs = constp.tile([128, KT * B], F32)
            nc.sync.dma_start(out=xTs[:], in_=xT)
            auxs = constp.tile([B, 8 * 128], F32)
            nc.sync.dma_start(out=auxs[:], in_=aux)
            Wos = constp.tile([128, 4 * HIDDEN], F32)
            nc.scalar.dma_start(out=Wos[:], in_=Wor)

            # ---- qkv projection: accumulate over 30 k-chunks --------------
            qkv_ps = pbig.tile([B, 1024], F32, tag="big")
            for g in range(KT // WG):
                wt = wqp.tile([128, WG * 1024], F32, tag="w")
                nc.sync.dma_start(
                    out=wt[:], in_=Wr[:, g * WG * 1024:(g + 1) * WG * 1024]
                )
                for j in range(WG):
                    kt = g * WG + j
                    lhsT = xTs[:, kt * B:(kt + 1) * B]
                    for nh in range(2):
                        nc.tensor.matmul(
                            qkv_ps[:, nh * 512:(nh + 1) * 512],
                            lhsT,
                            wt[:, j * 1024 + nh * 512: j * 1024 + (nh + 1) * 512],
                            start=(kt == 0),
                            stop=(kt == KT - 1),
                            skip_group_check=True,
                        )

            qkv_sb = constp.tile([B, 1024], F32)
            nc.scalar.copy(out=qkv_sb[:], in_=qkv_ps[:])

            # ---- per-head RMS norm factors (q h0, q h1, k) ----------------
            rs = []
            for i in range(3):
                sq = work.tile([B, HEAD_DIM], F32, tag="sq")
                ss = constp.tile([B, 1], F32, tag=f"ss{i}")
                nc.scalar.activation(
                    out=sq[:],
                    in_=qkv_sb[:, i * HEAD_DIM:(i + 1) * HEAD_DIM],
                    func=mybir.ActivationFunctionType.Square,
                    accum_out=ss[:],
                )
                std = constp.tile([B, 1], F32, tag=f"std{i}")
                nc.scalar.activation(
                    out=std[:], in_=ss[:],
                    func=mybir.ActivationFunctionType.Sqrt,
                    scale=1.0 / HEAD_DIM, bias=eps_t[:],
                )
                r = constp.tile([B, 1], F32, tag=f"rs{i}")
                nc.vector.reciprocal(out=r[:], in_=std[:])
                rs.append(r)

            # ---- rope + norm-weight fold ---------------------------------
            # aux cols: qc1 qs1 qc2 qs2 kc1 ks1 kc2 ks2 (each 128 wide)
            q2 = constp.tile([B, 1024], F32)
            for i in range(3):  # 0,1: q heads (SCALE folded in aux); 2: k
                a0 = 0 if i < 2 else 4
                x1 = qkv_sb[:, i * 256:i * 256 + 128]
                x2 = qkv_sb[:, i * 256 + 128:i * 256 + 256]
                c1 = auxs[:, (a0 + 0) * 128:(a0 + 1) * 128]
                s1 = auxs[:, (a0 + 1) * 128:(a0 + 2) * 128]
                c2 = auxs[:, (a0 + 2) * 128:(a0 + 3) * 128]
                s2 = auxs[:, (a0 + 3) * 128:(a0 + 4) * 128]
                t1 = work.tile([B, 128], F32, tag="t1")
                t2 = work.tile([B, 128], F32, tag="t2")
                nc.vector.tensor_mul(out=t1[:], in0=x1, in1=c1)
                nc.vector.tensor_mul(out=t2[:], in0=x2, in1=s2)
                nc.vector.tensor_sub(
                    out=q2[:, i * 256:i * 256 + 128], in0=t1[:], in1=t2[:]
                )
                t3 = work.tile([B, 128], F32, tag="t1")
                t4 = work.tile([B, 128], F32, tag="t2")
                nc.vector.tensor_mul(out=t3[:], in0=x2, in1=c2)
                nc.vector.tensor_mul(out=t4[:], in0=x1, in1=s1)
                nc.vector.tensor_add(
                    out=q2[:, i * 256 + 128:i * 256 + 256], in0=t3[:], in1=t4[:]
                )
                nc.vector.tensor_scalar_mul(
                    out=q2[:, i * 256:(i + 1) * 256],
                    in0=q2[:, i * 256:(i + 1) * 256],
                    scalar1=rs[i][:],
                )
            nc.vector.tensor_copy(out=q2[:, 768:1024], in_=qkv_sb[:, 768:1024])
            v_r = constp.tile([B, HEAD_DIM], F32R)
            nc.vector.tensor_copy(out=v_r[:], in_=q2[:, 768:1024])

            nc.gpsimd.dma_start(out=knew, in_=q2[:, 512:768])
            nc.gpsimd.dma_start(out=vnew, in_=q2[:, 768:1024])

            # ---- transposes: qT2 (interleaved (b,h)), kTn ----------------
            qT2 = []
            for dc in range(2):
                qt = constp.tile([128, 2 * B], F32R, tag=f"qT2{dc}")
                for h in range(2):
                    tp = psmall.tile([128, B], F32, tag="small")
                    nc.tensor.transpose(
                        tp[:], q2[:, h * 256 + dc * 128:h * 256 + (dc + 1) * 128],
                        ident[:B, :B],
                    )
                    nc.vector.tensor_copy(out=qt[:, h::2], in_=tp[:])
                qT2.append(qt)
            kTn = constp.tile([128, 2 * B], F32R)  # cols dc*32+b
            for dc in range(2):
                tp = psmall.tile([128, B], F32, tag="small")
                nc.tensor.transpose(
                    tp[:], q2[:, 512 + dc * 128:512 + (dc + 1) * 128],
                    ident[:B, :B],
                )
                nc.vector.tensor_copy(out=kTn[:, dc * B:(dc + 1) * B], in_=tp[:])

            # ---- attention over 16 groups of 2 batches -------------------
            # PE psum writes must start at partition 0/32/64: batch bi of a
            # group uses psum rows 32*bi .. 32*bi+1 (rows 2..31 are unused
            # garbage that never mixes into valid rows).
            attnT = []  # per dc: (128, 64) cols 2b+h (interleaved)
            for dc in range(2):
                at = constp.tile([128, 2 * B], F32, tag=f"attnTi{dc}")
                attnT.append(at)
            for g in range(NG):
                Kt = kvp.tile([128, GB * 2 * S], F32R, tag="K")  # (dc,bi,s)
                nc.sync.dma_start(out=Kt[:], in_=KTc[g])
                Vt = kvp.tile([128, GB * 8 * HEAD_DIM], F32R, tag="V")  # (st,bi,d)
                nc.scalar.dma_start(out=Vt[:], in_=Vrc[g])

                # insert new k column (write_idx) for both dc, both batches
                for dc in range(2):
                    nc.vector.tensor_copy(
                        out=Kt[:, dc * 2 * S + write_idx: dc * 2 * S + write_idx + S + 1: S],
                        in_=kTn[:, dc * B + GB * g: dc * B + GB * (g + 1)],
                    )
                # insert new v row (v_r is the f32r-rounded copy of v')
                nc.gpsimd.dma_start(
                    out=Vt[p_w:p_w + 1, st_w * GB * HEAD_DIM:(st_w + 1) * GB * HEAD_DIM],
                    in_=v_r[GB * g:GB * (g + 1), :],
                )

                for bi in range(GB):
                    scr = pbig.tile([2, S], F32, tag="big")
                    for dc in range(2):
                        for nh in range(2):
                            nc.tensor.matmul(
                                scr[0:2, nh * 512:(nh + 1) * 512],
                                qT2[dc][:, 2 * (GB * g + bi):2 * (GB * g + bi) + 2],
                                Kt[:, dc * 2 * S + bi * S + nh * 512: dc * 2 * S + bi * S + (nh + 1) * 512],
                                start=(dc == 0), stop=(dc == 1),
                                skip_group_check=True,
                            )

                    negmax = work.tile([2, 1], F32, tag="negmax")
                    nc.vector.tensor_reduce(
                        out=negmax[:], in_=scr[:],
                        axis=mybir.AxisListType.X, op=mybir.AluOpType.max, negate=True,
                    )
                    probs = work.tile([2, S], F32, tag="probs")
                    sumexp = work.tile([2, 1], F32, tag="sumexp")
                    nc.scalar.activation(
                        out=probs[:], in_=scr[:],
                        func=mybir.ActivationFunctionType.Exp,
                        bias=negmax[:], accum_out=sumexp[:],
                    )
                    rcp = work.tile([2, 1], F32, tag="rcp")
                    nc.vector.reciprocal(out=rcp[:], in_=sumexp[:])
                    nc.vector.tensor_scalar_mul(
                        out=probs[:], in0=probs[:], scalar1=rcp[:]
                    )

                    pTb = work.tile([128, 16], F32R, tag="pT")
                    for st in range(8):
                        tp = psmall.tile([128, 2], F32, tag="small")
                        nc.tensor.transpose(
                            tp[:], probs[:, st * 128:(st + 1) * 128],
                            ident[:2, :2],
                        )
                        nc.vector.tensor_copy(
                            out=pTb[:, st * 2:(st + 1) * 2], in_=tp[:]
                        )

                    attn_b = pattn.tile([2, HEAD_DIM], F32, tag="attn")
                    for st in range(8):
                        nc.tensor.matmul(
                            attn_b[0:2, :],
                            pTb[:, st * 2:st * 2 + 2],
                            Vt[:, st * GB * HEAD_DIM + bi * HEAD_DIM: st * GB * HEAD_DIM + (bi + 1) * HEAD_DIM],
                            start=(st == 0), stop=(st == 7),
                            skip_group_check=True,
                        )

                    attn_sg = work.tile([2, HEAD_DIM], F32, tag="attnsg")
                    nc.scalar.copy(out=attn_sg[:], in_=attn_b[:])
                    for dc in range(2):
                        tp = psmall.tile([128, 2], F32, tag="small")
                        nc.tensor.transpose(
                            tp[:], attn_sg[:, dc * 128:(dc + 1) * 128], ident[:2, :2]
                        )
                        nc.vector.tensor_copy(
                            out=attnT[dc][:, 2 * (GB * g + bi):2 * (GB * g + bi) + 2],
                            in_=tp[:],
                        )

            # ---- output projection ---------------------------------------
            attnD = []  # dci = h*2+dc -> (128, 32) cols b
            for dci in range(4):
                h, dc = divmod(dci, 2)
                at = constp.tile([128, B], F32, tag=f"attnT{dci}")
                nc.vector.tensor_copy(out=at[:], in_=attnT[dc][:, h::2])
                attnD.append(at)

            y_sb = wqp.tile([B, HIDDEN], F32, tag="w")
            off = 0
            while off < HIDDEN:
                w = min(512, HIDDEN - off)
                y_ps = pbig.tile([B, 512], F32, tag="big")
                for dci in range(4):
                    nc.tensor.matmul(
                        y_ps[:, :w],
                        attnD[dci][:],
                        Wos[:, dci * HIDDEN + off: dci * HIDDEN + off + w],
                        start=(dci == 0), stop=(dci == 3),
                        skip_group_check=True,
                    )
                nc.scalar.copy(out=y_sb[:, off:off + w], in_=y_ps[:, :w])
                off += w
            nc.gpsimd.dma_start(out=y, in_=y_sb[:])

    return nc


_CACHE = {}


def _get_nc(write_idx):
    if write_idx not in _CACHE:
        _CACHE[write_idx] = build_kernel(write_idx)
    return _CACHE[write_idx]


def kernel(x, Wq, Wk, Wv, Wo, q_norm_w, k_norm_w, k_cache, v_cache,
           pos_offset, cache_index, _want_trace=False):
    x2 = np.asarray(x, np.float32).reshape(B, HIDDEN)
    write_idx = int(cache_index) % S
    valid_len = min(int(cache_index) + 1, S)
    assert valid_len == S, "kernel assumes a fully-populated sliding window"

    # x transposed+tiled: xTr[p, t*32+b] = x2[b, t*128+p]
    xTr = np.ascontiguousarray(
        x2.T.reshape(KT, 128, B).transpose(1, 0, 2).reshape(128, KT * B)
    )

    # rope tables with norm weights (+ attention scale for q) folded in
    freqs = ROPE_BASE ** (-np.arange(0, HEAD_DIM, 2, dtype=np.float32) / HEAD_DIM)
    theta = np.float32(pos_offset) * freqs
    cos, sin = np.cos(theta, dtype=np.float32), np.sin(theta, dtype=np.float32)
    qw1 = 1.0 + np.asarray(q_norm_w, np.float32)[:128]
    qw2 = 1.0 + np.asarray(q_norm_w, np.float32)[128:]
    kw1 = 1.0 + np.asarray(k_norm_w, np.float32)[:128]
    kw2 = 1.0 + np.asarray(k_norm_w, np.float32)[128:]
    vecs = [qw1 * cos * SCALE, qw1 * sin * SCALE, qw2 * cos * SCALE,
            qw2 * sin * SCALE, kw1 * cos, kw1 * sin, kw2 * cos, kw2 * sin]
    aux = np.ascontiguousarray(
        np.tile(np.concatenate(vecs).astype(np.float32)[None, :], (B, 1))
    )

    # cache relayouts (pure glue: same bytes, DMA-friendly order)
    kc = np.asarray(k_cache, np.float32)
    vc = np.asarray(v_cache, np.float32)
    # KT_all[h, g, p, (dc,bi,s)]
    KT_all = np.ascontiguousarray(
        kc.reshape(NG, GB, NUM_KV_HEADS, S, 2, 128)
          .transpose(2, 0, 5, 4, 1, 3).reshape(NUM_KV_HEADS, NG, 128, GB * 2 * S)
    )
    # V_all[h, g, p, (st,bi,d)]
    V_all = np.ascontiguousarray(
        vc.reshape(NG, GB, NUM_KV_HEADS, 8, 128, HEAD_DIM)
          .transpose(2, 0, 4, 3, 1, 5).reshape(NUM_KV_HEADS, NG, 128, GB * 8 * HEAD_DIM)
    )

    Wq_ = np.asarray(Wq, np.float32)
    Wk_ = np.asarray(Wk, np.float32)
    Wv_ = np.asarray(Wv, np.float32)
    Wo_ = np.asarray(Wo, np.float32)

    in_maps = []
    for c in range(N_CORES):
        Wqkv_c = np.concatenate(
            [Wq_[:, c * QCOLS:(c + 1) * QCOLS],
             Wk_[:, c * HEAD_DIM:(c + 1) * HEAD_DIM],
             Wv_[:, c * HEAD_DIM:(c + 1) * HEAD_DIM]], axis=1)
        Wr_c = np.ascontiguousarray(
            Wqkv_c.reshape(KT, 128, 1024).transpose(1, 0, 2).reshape(128, KT * 1024)
        )
        Wo_c = np.ascontiguousarray(
            Wo_[c * QCOLS:(c + 1) * QCOLS, :]
            .reshape(4, 128, HIDDEN).transpose(1, 0, 2).reshape(128, 4 * HIDDEN)
        )
        in_maps.append({
            "xT": xTr, "aux": aux, "Wr": Wr_c, "Wor": Wo_c,
            "KTc": np.ascontiguousarray(KT_all[c]),
            "Vrc": np.ascontiguousarray(V_all[c]),
        })

    nc = _get_nc(write_idx)
    res = run_bass_kernel_spmd(
        nc, in_maps, core_ids=list(range(N_CORES)), trace=_want_trace
    )

    y = np.zeros((B, HIDDEN), np.float64)
    for c in range(N_CORES):
        y += res.results[c]["y"]
    y = y.astype(np.float32).reshape(B, 1, HIDDEN)

    k_out = kc.copy()
    v_out = vc.copy()
    for c in range(N_CORES):
        k_out[:, c, write_idx, :] = res.results[c]["knew"]
        v_out[:, c, write_idx, :] = res.results[c]["vnew"]

    if _want_trace:
        kernel.last_results = res
    return y, k_out, v_out
